# revision 20
# baseline (speedup 1.0000x reference)
"""Deformable cross-attention Trainium2 kernel (8-core SPMD, query-sharded).

Strategy (v2)
-------------
q_len = 64*64 = 4096 BEV queries split across 8 cores (512 each).  Per core:
  1. kv conv (PE, fp32) -> bf16 "kv2" scratch per camera in HBM:
     row r = y*88+x holds 1024 ch = [row r: k 256 | v 256][row r+88: k | v],
     i.e. the y+1 row is stacked channel-wise so ONE 4KB gather element
     (2 consecutive x positions) fetches the whole 2x2 bilinear footprint.
  2. All projections / offsets / coords / gather indices for all
     4 chunks x 6 cams computed upfront in a few batched DVE ops.
  3. Per (cam, chunk): dma_gather 1024 elements (128 q x 8 pts) of 4KB,
     then attention in bf16: q.k products (2x DVE mode), contiguous
     halving tree-adds for the dh-reduction (channels stored d-major so
     head lanes stay innermost/packed), score bilinear interp, softmax
     over points, a4 = att*wx*wy/6 folded weights, v weighted tree-sum.
  4. Output projection on PE per chunk.
Channels are permuted d-major (j = d*8+m <- m*32+d) host-side in
q_w/kv_w/proj_w so device reductions over d are contiguous halves.
No collectives; host concatenates the 8 query slices.

Free-dim biases q_b, kv_b, off_b2 are zeros per spec and not applied.
"""

import sys

for _p in ("/opt/trn_rl_repo", "/opt/trn_rl_repo/concourse"):
    if _p not in sys.path:
        sys.path.insert(0, _p)

from contextlib import ExitStack

import numpy as np

import concourse.bass as bass
import concourse.mybir as mybir
import concourse.tile as tile
from concourse import bacc, library_config
from concourse.bass_utils import run_bass_kernel_spmd

F32 = mybir.dt.float32
BF16 = mybir.dt.bfloat16
I16 = mybir.dt.int16
ALU = mybir.AluOpType
ACTF = mybir.ActivationFunctionType
AX = mybir.AxisListType

N_CORES = 8
D = 128
N_CAM = 6
H_BEV, W_BEV = 64, 64
Q_LEN = H_BEV * W_BEV            # 4096
QC = Q_LEN // N_CORES            # 512
N_CHUNK = QC // 128              # 4
HEADS, DH, NPTS = 8, 32, 8
INNER = HEADS * DH               # 256
HI, WI = 32, 88
POS = HI * WI                    # 2816
NPB = POS // 128                 # 22
CH2 = 1024                       # stacked kv2 channels per row
NIT = N_CHUNK * N_CAM            # 24 (cam, chunk) pairs

_PROGRAM = None


def _build_program():
    nc = bacc.Bacc("TRN2", target_bir_lowering=False, debug=False,
                   num_swdge_queues=2)

    # ---------------- I/O ----------------
    t_bev = nc.dram_tensor("bev_s", [D, QC], F32, kind="ExternalInput")
    t_world = nc.dram_tensor("world_s", [4, QC], F32, kind="ExternalInput")
    t_img = nc.dram_tensor("img", [N_CAM, D, POS], F32, kind="ExternalInput")
    t_mt = nc.dram_tensor("MT", [4, 3 * N_CAM], F32, kind="ExternalInput")
    t_w1T = nc.dram_tensor("w1T", [D, D], F32, kind="ExternalInput")
    t_w2T = nc.dram_tensor("w2T", [D, 2 * NPTS], F32, kind="ExternalInput")
    t_qwT = nc.dram_tensor("qwT", [D, INNER], F32, kind="ExternalInput")
    t_kvwT = nc.dram_tensor("kvwT", [D, 2 * INNER], F32, kind="ExternalInput")
    t_pwT = nc.dram_tensor("pwT", [128, 2, D], F32, kind="ExternalInput")
    t_b1 = nc.dram_tensor("b1", [D, 1], F32, kind="ExternalInput")
    t_pb = nc.dram_tensor("pb", [D, 1], F32, kind="ExternalInput")
    t_sel = nc.dram_tensor("selW", [128, 128], F32, kind="ExternalInput")
    t_mask = nc.dram_tensor("maskW", [128, 8], F32, kind="ExternalInput")
    t_idn = nc.dram_tensor("idn", [128, 128], F32, kind="ExternalInput")
    t_out = nc.dram_tensor("out", [D, QC], F32, kind="ExternalOutput")

    with tile.TileContext(nc) as tc, ExitStack() as ctx:
        nc.gpsimd.load_library(library_config.mlp)

        consts = ctx.enter_context(tc.tile_pool(name="consts", bufs=1))
        setupp = ctx.enter_context(tc.tile_pool(name="setup", bufs=1))
        drampool = ctx.enter_context(tc.tile_pool(name="dram", bufs=1, space="DRAM"))
        psA = ctx.enter_context(tc.tile_pool(name="psA", bufs=3, space="PSUM"))
        p2ps = psA
        p1ps = psA
        outps = psA

        def load_const(t, shape):
            s = consts.tile(shape, F32, tag=t.name)
            nc.sync.dma_start(s[:], t.ap())
            return s

        mt_all = load_const(t_mt, [4, 3 * N_CAM])
        xyz = load_const(t_world, [4, QC])
        c_bev = load_const(t_bev, [D, QC])
        c_w1T = load_const(t_w1T, [D, D])
        c_b1 = load_const(t_b1, [D, 1])
        c_w2T = load_const(t_w2T, [D, 2 * NPTS])
        c_kvwT = load_const(t_kvwT, [D, 2 * INNER])
        c_mask = load_const(t_mask, [128, 8])
        c_sel = load_const(t_sel, [128, 128])
        c_qwT = load_const(t_qwT, [D, INNER])
        c_pwT = load_const(t_pwT, [128, 2, D])
        c_pb = load_const(t_pb, [D, 1])
        c_idn = load_const(t_idn, [128, 128])

        kv2 = [drampool.tile([POS, CH2], BF16, tag=f"kv2_{n}", name=f"kv2_{n}")
               for n in range(N_CAM)]

        # ---------------- P2a: PE projections ----------------
        xh = setupp.tile([D, QC], F32, tag="xh_masked")
        pix_all = setupp.tile([128, N_CHUNK, 3 * N_CAM], F32)
        offT_all = setupp.tile([128, N_CHUNK, 2 * NPTS], F32)
        qT_all = setupp.tile([128, N_CHUNK, INNER], BF16)

        ps_xh = p2ps.tile([D, QC], F32, tag="big")
        nc.tensor.matmul(ps_xh[:], c_w1T[:], c_bev[:], start=True, stop=True)
        nc.scalar.activation(xh[:], ps_xh[:], ACTF.Relu, bias=c_b1[:])
        for c in range(N_CHUNK):
            cs = slice(c * 128, (c + 1) * 128)
            ps_pix = p2ps.tile([128, 3 * N_CAM], F32, tag="sm")
            nc.tensor.matmul(ps_pix[:], xyz[:, cs], mt_all[:], start=True, stop=True)
            nc.scalar.copy(pix_all[:, c, :], ps_pix[:])
        for c in range(N_CHUNK):
            cs = slice(c * 128, (c + 1) * 128)
            ps_o = p2ps.tile([128, 2 * NPTS], F32, tag="sm")
            nc.tensor.matmul(ps_o[:], xh[:, cs], c_w2T[:], start=True, stop=True)
            nc.scalar.copy(offT_all[:, c, :], ps_o[:])
        for c in range(N_CHUNK):
            cs = slice(c * 128, (c + 1) * 128)
            ps_q = p2ps.tile([128, INNER], F32, tag="sm")
            nc.tensor.matmul(ps_q[:], c_bev[:, cs], c_qwT[:], start=True, stop=True)
            nc.scalar.copy(qT_all[:, c, :], ps_q[:])

        # ---------------- P1: kv conv -> bf16 kv2 scratch ----------------
        p1pool = ctx.enter_context(tc.tile_pool(name="p1", bufs=1))
        CONV_PIECES = [range(0, 8), range(8, 16), range(16, NPB)]

        def conv_load(n):
            img_t = p1pool.tile([D, POS], F32, tag="img", name="img_t", bufs=2)
            nc.sync.dma_start(img_t[:], t_img.ap()[n])
            return img_t

        def conv_piece(n, img_t, blocks):
            # compute blocks [b0, b1) and store them to kv2[n] immediately
            b0, b1 = blocks.start, blocks.stop
            nb = b1 - b0
            stg = p1pool.tile([128, 8, 2 * INNER], BF16, tag="stg", name="stg", bufs=2)
            for k, pb in enumerate(blocks):
                ps = p1ps.tile([128, 2 * INNER], F32, tag="big", name="ps")
                nc.tensor.matmul(
                    ps[:], img_t[:, pb * 128:(pb + 1) * 128], c_kvwT[:],
                    start=True, stop=True)
                nc.scalar.copy(stg[:, k, :], ps[:])
            # rows r=pb*128+p -> kv2[r, 0:512]
            dst = bass.AP(kv2[n][:].tensor, b0 * 128 * CH2,
                          [[CH2, 128], [128 * CH2, nb], [1, 512]])
            nc.scalar.dma_start(dst, stg[:, 0:nb, :])
            # shifted copy: kv2[r-88, 512:1024] = row r  (r >= 88)
            if b0 == 0:
                dst_a = bass.AP(kv2[n][:].tensor, 512, [[CH2, 40], [1, 512]])
                nc.scalar.dma_start(dst_a, stg[88:128, 0, :])
                dst_b = bass.AP(kv2[n][:].tensor, 40 * CH2 + 512,
                                [[CH2, 128], [128 * CH2, nb - 1], [1, 512]])
                nc.scalar.dma_start(dst_b, stg[:, 1:nb, :])
            else:
                dst_b = bass.AP(kv2[n][:].tensor, (b0 * 128 - 88) * CH2 + 512,
                                [[CH2, 128], [128 * CH2, nb], [1, 512]])
                nc.scalar.dma_start(dst_b, stg[:, 0:nb, :])

        img_m = conv_load(0)
        img1 = conv_load(1)
        for piece in CONV_PIECES:
            conv_piece(0, img_m, piece)

        # ---------------- P2b: coords / indices (batched DVE) ----------------
        NCN = N_CHUNK * N_CAM            # 24
        NQP = NCN * NPTS                 # 192
        sm24 = setupp.tile([128, 2 * NCN], F32)      # [zr rz | ux uy | gx gy]
        gx = setupp.tile([128, NCN], F32)
        gy = setupp.tile([128, NCN], F32)
        xw = setupp.tile([128, NQP], F32, tag="xw_shared")
        xs = setupp.tile([128, NQP], F32, tag="xs_shared")
        x0f = setupp.tile([128, NQP], F32)
        wxp = setupp.tile([128, NQP], F32)
        yw = setupp.tile([128, NQP], F32, tag="xw_shared")
        ys_ = setupp.tile([128, NQP], F32, tag="xs_shared")
        y0f = setupp.tile([128, NQP], F32)
        wyp = setupp.tile([128, NQP], F32)
        gtt = setupp.tile([128, NQP], F32)
        i16t = setupp.tile([128, NQP], I16)
        wx2 = setupp.tile([128, NQP, 2], F32)
        wy2 = setupp.tile([128, NQP, 2], F32)
        wxyk = setupp.tile([128, NQP, 2, 2], F32)
        wxy = setupp.tile([128, NQP, 2, 2], F32)
        i128 = setupp.tile([128, NQP], F32)
        masked = setupp.tile([128, 8, NPTS, 8], F32, tag="xh_masked")
        wrapped = setupp.tile([128, NCN, 64], I16)

        pixv = pix_all[:].rearrange("P c (n k) -> P c n k", n=N_CAM)
        zr = sm24[:, 0:NCN].rearrange("P (c n) -> P c n", c=N_CHUNK)
        rz = sm24[:, NCN:2 * NCN].rearrange("P (c n) -> P c n", c=N_CHUNK)
        nc.vector.tensor_scalar_max(zr, pixv[:, :, :, 2], 1e-6)
        nc.vector.reciprocal(rz, zr)
        gxv = gx[:].rearrange("P (c n) -> P c n", c=N_CHUNK)
        gyv = gy[:].rearrange("P (c n) -> P c n", c=N_CHUNK)
        nc.vector.tensor_mul(gxv, pixv[:, :, :, 0], rz)
        nc.vector.tensor_scalar(gxv, gxv, 2.0 / (WI - 1), -1.0, ALU.mult, ALU.add)
        nc.vector.tensor_mul(gyv, pixv[:, :, :, 1], rz)
        nc.vector.tensor_scalar(gyv, gyv, 2.0 / (HI - 1), -1.0, ALU.mult, ALU.add)

        offv = offT_all[:].rearrange("P c (p a) -> P c a p", a=2)

        def coord_chain(g_t, off_ax, w_t, s_t, f0_t, wfrac_t, hi_clip, scale):
            # w = clip(g + off, -1, 1) * scale + scale ; floor/clamp -> f0, frac
            wv = w_t[:].rearrange("P (c n p) -> P c n p", c=N_CHUNK, n=N_CAM)
            gb = g_t[:].rearrange("P (c n) -> P c n", c=N_CHUNK) \
                .unsqueeze(3).broadcast_to((128, N_CHUNK, N_CAM, NPTS))
            ob = offv[:, :, off_ax, :].unsqueeze(2) \
                .broadcast_to((128, N_CHUNK, N_CAM, NPTS))
            nc.vector.tensor_tensor(wv, gb, ob, ALU.add)
            nc.vector.tensor_scalar_min(w_t[:], w_t[:], 1.0)
            nc.vector.tensor_scalar_max(w_t[:], w_t[:], -1.0)
            nc.vector.tensor_scalar(w_t[:], w_t[:], scale, scale, ALU.mult, ALU.add)
            nc.vector.tensor_scalar_min(s_t[:], w_t[:], hi_clip)
            nc.vector.tensor_copy(i16t[:], s_t[:])
            nc.vector.tensor_copy(f0_t[:], i16t[:])
            nc.vector.tensor_tensor(gtt[:], f0_t[:], s_t[:], ALU.is_gt)
            nc.vector.tensor_sub(f0_t[:], f0_t[:], gtt[:])
            nc.vector.tensor_sub(wfrac_t[:], w_t[:], f0_t[:])

        coord_chain(gx, 0, xw, xs, x0f, wxp, float(WI - 2) + 0.5, (WI - 1) / 2.0)
        coord_chain(gy, 1, yw, ys_, y0f, wyp, float(HI - 2) + 0.5, (HI - 1) / 2.0)

        # gather row index = y0*88 + x0 (camera-local)
        nc.vector.tensor_scalar(i128[:], y0f[:], float(WI), 0.0, ALU.mult, ALU.add)
        nc.vector.tensor_add(i128[:], i128[:], x0f[:])

        # wrap indices for SWDGE: wrapped[r, it, pt*8+c8] = i128[c8*16+r, it, pt]
        i128v = i128[:].rearrange("P (i p) -> P i p", i=NCN)
        wflat = wrapped[:].rearrange("P i w -> P (i w)")
        for b in range(3):
            nc.vector.tensor_mul(
                masked[:],
                i128v[:, b * 8:(b + 1) * 8, :]
                .unsqueeze(3).broadcast_to((128, 8, NPTS, 8)),
                c_mask[:].unsqueeze(1).unsqueeze(2)
                .broadcast_to((128, 8, NPTS, 8)))
            ps_w = p2ps.tile([128, 512], F32, tag="big")
            nc.tensor.matmul(
                ps_w[:], c_sel[:],
                masked[:].rearrange("P i p e -> P (i p e)"),
                start=True, stop=True)
            nc.vector.tensor_copy(wflat[:, b * 512:(b + 1) * 512], ps_w[:])

        # corner weight products (1/N_CAM folded into wxy)
        nc.vector.tensor_scalar(wx2[:, :, 0], wxp[:], -1.0, 1.0, ALU.mult, ALU.add)
        nc.vector.tensor_copy(wx2[:, :, 1], wxp[:])
        nc.vector.tensor_scalar(wy2[:, :, 0], wyp[:], -1.0, 1.0, ALU.mult, ALU.add)
        nc.vector.tensor_copy(wy2[:, :, 1], wyp[:])
        nc.vector.tensor_mul(
            wxyk[:],
            wx2[:].unsqueeze(3).broadcast_to((128, NQP, 2, 2)),
            wy2[:].unsqueeze(2).broadcast_to((128, NQP, 2, 2)))
        nc.vector.tensor_scalar(wxy[:], wxyk[:], 1.0 / N_CAM, 0.0, ALU.mult, ALU.add)

        for piece in CONV_PIECES:
            conv_piece(1, img1, piece)

        # ---------------- P3: gather + attention ----------------
        gpool = ctx.enter_context(tc.tile_pool(name="G", bufs=3))
        kpool = ctx.enter_context(tc.tile_pool(name="kv", bufs=1))
        spool = ctx.enter_context(tc.tile_pool(name="small", bufs=2))
        accp = ctx.enter_context(tc.tile_pool(name="acc", bufs=1))

        accs = [accp.tile([128, INNER], F32, tag=f"acc{c}", name=f"acc{c}")
                for c in range(N_CHUNK)]
        wyv_all = wyp[:].rearrange("P (c n p) -> P c n p", c=N_CHUNK, n=N_CAM)
        wxv_all = wxp[:].rearrange("P (c n p) -> P c n p", c=N_CHUNK, n=N_CAM)
        wxyv_all = wxy[:].rearrange(
            "P (c n p) a b -> P c n (p a b)", c=N_CHUNK, n=N_CAM)
        wxyk_all = wxyk[:].rearrange(
            "P (c n p) a b -> P c n (p a b)", c=N_CHUNK, n=N_CAM)

        nxt = [None]

        for n in range(N_CAM):
            kv_src = bass.AP(kv2[n][:].tensor, 0, [[CH2, POS - 1], [1, 2048]])
            for c in range(N_CHUNK):
                it = c * N_CAM + n
                g = gpool.tile([128, NPTS, 2048], BF16, tag="G")
                nc.gpsimd.dma_gather(
                    g[:], kv_src,
                    wrapped[:, it, :], 1024, 1024,
                    elem_size=2048, elem_step=CH2, single_packet=True,
                    queue_num=it % 2)
                gkv = g[:].rearrange(
                    "P b (x y k i) -> P (b x y) k i", x=2, y=2, k=2)
                # ---- k side: prod = k * q, tree-reduce over d ----
                prod = kpool.tile([128, 32, INNER], BF16, tag="prod")
                nc.vector.tensor_mul(
                    prod[:], gkv[:, :, 0, :],
                    qT_all[:, c, :].unsqueeze(1).broadcast_to((128, 32, INNER)))
                t1 = kpool.tile([128, 32, 128], BF16, tag="t1")
                nc.vector.tensor_add(t1[:], prod[:, :, 0:128], prod[:, :, 128:256])
                t2 = kpool.tile([128, 32, 64], BF16, tag="t2")
                nc.vector.tensor_add(t2[:], t1[:, :, 0:64], t1[:, :, 64:128])
                t3 = kpool.tile([128, 32, 32], BF16, tag="t3")
                nc.vector.tensor_add(t3[:], t2[:, :, 0:32], t2[:, :, 32:64])
                t4 = kpool.tile([128, 32, 16], BF16, tag="t4")
                nc.vector.tensor_add(t4[:], t3[:, :, 0:16], t3[:, :, 16:32])

                # ---- bilinear interp of corner scores (weighted sum),
                #      fused with the last d-pair reduction level ----
                sims = kpool.tile([128, 32, 16], BF16, tag="sims")
                nc.vector.tensor_mul(
                    sims[:].rearrange("P s (d m) -> P s d m", m=HEADS),
                    t4[:].rearrange("P s (d m) -> P s d m", m=HEADS),
                    wxyk_all[:, c, n, :].unsqueeze(2).unsqueeze(3)
                    .broadcast_to((128, 32, 2, HEADS)))
                sim = spool.tile([128, NPTS, HEADS], F32, tag="sim")
                nc.vector.tensor_reduce(
                    sim[:],
                    sims[:].rearrange("P (p s) (d m) -> P p m (s d)",
                                      s=4, m=HEADS),
                    AX.X, ALU.add)

                # ---- softmax over points (scores bounded, skip max-sub) ----
                ev = spool.tile([128, NPTS, HEADS], F32, tag="ev")
                nc.scalar.activation(ev[:], sim[:], ACTF.Exp)
                ssum = spool.tile([128, HEADS], F32, tag="ssum")
                nc.vector.tensor_reduce(
                    ssum[:], ev[:].transpose([0, 2, 1]), AX.X, ALU.add)
                rr = spool.tile([128, HEADS], F32, tag="rr")
                nc.vector.reciprocal(rr[:], ssum[:])
                att = spool.tile([128, NPTS, HEADS], F32, tag="att")
                nc.vector.tensor_mul(
                    att[:], ev[:],
                    rr[:].unsqueeze(1).broadcast_to((128, NPTS, HEADS)))

                # ---- a4 = att * wx * wy / n  (bf16, [q, slot, m]) ----
                a4 = spool.tile([128, 32, HEADS], BF16, tag="a4")
                a4v = a4[:].rearrange("P (p s) m -> P p s m", s=4)
                nc.vector.tensor_mul(
                    a4v,
                    att[:].unsqueeze(2).broadcast_to((128, NPTS, 4, HEADS)),
                    wxyv_all[:, c, n, :].rearrange("P (p s) -> P p s", p=NPTS)
                    .unsqueeze(3).broadcast_to((128, NPTS, 4, HEADS)))

                # ---- v side: weighted tree-sum over 32 corner slots ----
                prodv = kpool.tile([128, 32, INNER], BF16, tag="prod")
                nc.vector.tensor_mul(
                    prodv[:].rearrange("P s (d m) -> P s d m", m=HEADS),
                    gkv[:, :, 1, :].rearrange("P s (d m) -> P s d m", m=HEADS),
                    a4[:].unsqueeze(2).broadcast_to((128, 32, DH, HEADS)))
                v1 = kpool.tile([128, 16, INNER], BF16, tag="t1")
                nc.vector.tensor_add(v1[:], prodv[:, 0:16, :], prodv[:, 16:32, :])
                v2 = kpool.tile([128, 8, INNER], BF16, tag="t2")
                nc.vector.tensor_add(v2[:], v1[:, 0:8, :], v1[:, 8:16, :])
                v3 = kpool.tile([128, 4, INNER], BF16, tag="t3")
                nc.vector.tensor_add(v3[:], v2[:, 0:4, :], v2[:, 4:8, :])
                v4 = kpool.tile([128, 2, INNER], BF16, tag="t4")
                nc.vector.tensor_add(v4[:], v3[:, 0:2, :], v3[:, 2:4, :])
                if n == 0:
                    nc.vector.tensor_add(accs[c][:], v4[:, 0, :], v4[:, 1, :])
                else:
                    v5 = kpool.tile([128, INNER], F32, tag="simc")
                    nc.vector.tensor_add(v5[:], v4[:, 0, :], v4[:, 1, :])
                    nc.vector.tensor_add(accs[c][:], accs[c][:], v5[:])

                # trickle cam n+2's conv with a full-camera lead so its
                # kv2 stores land well before that camera's gathers (emitted
                # after the attention body so exp precedes the copies in the
                # Scalar queue)
                if n + 2 < N_CAM and c < 3:
                    if c == 0:
                        nxt[0] = conv_load(n + 2)
                    conv_piece(n + 2, nxt[0], CONV_PIECES[c])

                # ---- P4: output projection, interleaved on the last cam ----
                if n == N_CAM - 1:
                    ps_out = outps.tile([128, 128], F32, tag="big")
                    for hh in range(2):
                        ps_tr = outps.tile([128, 128], F32, tag="sm")
                        nc.tensor.transpose(
                            ps_tr[:], accs[c][:, hh * 128:(hh + 1) * 128],
                            c_idn[:])
                        accT = kpool.tile([128, 128], F32, tag="t1")
                        nc.scalar.copy(accT[:], ps_tr[:])
                        nc.tensor.matmul(
                            ps_out[:], c_pwT[:, hh, :], accT[:],
                            start=(hh == 0), stop=(hh == 1))
                    out_sb = kpool.tile([128, 128], F32, tag="t2")
                    nc.vector.tensor_scalar_add(out_sb[:], ps_out[:], c_pb[:])
                    nc.sync.dma_start(
                        t_out.ap()[:, c * 128:(c + 1) * 128], out_sb[:])

    nc.compile()
    return nc


def _get_program():
    global _PROGRAM
    if _PROGRAM is None:
        _PROGRAM = _build_program()
    return _PROGRAM


def _host_inputs(inputs):
    bev = np.asarray(inputs["bev"], np.float32)
    img_feats = np.asarray(inputs["img_feats"], np.float32)
    K = np.asarray(inputs["K"], np.float32)
    E = np.asarray(inputs["E"], np.float32)
    world_xy = np.asarray(inputs["world_xy"], np.float32)

    bev2 = np.ascontiguousarray(bev.reshape(D, Q_LEN))
    world2 = np.ascontiguousarray(world_xy.reshape(2, Q_LEN))
    img = np.ascontiguousarray(img_feats.reshape(N_CAM, D, POS))
    # MT[n] = (K[n] @ E[n][:3, :]).T : [4, 3] per camera, concatenated
    mt = np.einsum('nij,njk->nik', K[0], E[0][:, :3, :])  # [n, 3, 4]
    mt = np.ascontiguousarray(mt.transpose(2, 0, 1).reshape(4, 3 * N_CAM))

    # d-major channel permutation: new j = d*8 + m  <-  old m*32 + d
    j = np.arange(INNER)
    pm = (j % HEADS) * DH + j // HEADS

    w1T = np.ascontiguousarray(np.asarray(inputs["off_w1"], np.float32).T)
    w2T = np.ascontiguousarray(np.asarray(inputs["off_w2"], np.float32).T)
    qwT = np.asarray(inputs["q_w"], np.float32).T
    qwT = np.ascontiguousarray(qwT[:, pm])
    kvwT = np.asarray(inputs["kv_w"], np.float32).T
    kvwT = np.ascontiguousarray(
        np.concatenate([kvwT[:, :INNER][:, pm], kvwT[:, INNER:][:, pm]], axis=1))
    pwTp = np.asarray(inputs["proj_w"], np.float32).T[pm]
    pwT = np.ascontiguousarray(pwTp.reshape(2, 128, 128).transpose(1, 0, 2))
    b1 = np.ascontiguousarray(np.asarray(inputs["off_b1"], np.float32).reshape(D, 1))
    pb = np.ascontiguousarray(np.asarray(inputs["proj_b"], np.float32).reshape(D, 1))

    kk = np.arange(128)
    sel = (kk[:, None] % 16 == kk[None, :] % 16).astype(np.float32)
    mask = (kk[:, None] // 16 == np.arange(8)[None, :]).astype(np.float32)
    idn = np.eye(128, dtype=np.float32)

    shared = dict(img=img, MT=mt, w1T=w1T, w2T=w2T, qwT=qwT, kvwT=kvwT,
                  pwT=pwT, b1=b1, pb=pb, selW=sel, maskW=mask, idn=idn)
    maps = []
    for r in range(N_CORES):
        s = slice(r * QC, (r + 1) * QC)
        m = dict(shared)
        m["bev_s"] = np.ascontiguousarray(bev2[:, s])
        ws = np.empty((4, QC), np.float32)
        ws[0:2] = world2[:, s]
        ws[2] = 0.0
        ws[3] = 1.0
        m["world_s"] = ws
        maps.append(m)
    return maps


def kernel(**inputs) -> np.ndarray:
    nc = _get_program()
    maps = _host_inputs(inputs)
    res = run_bass_kernel_spmd(nc, maps, list(range(N_CORES)))
    out = np.concatenate([res.results[r]["out"] for r in range(N_CORES)], axis=1)
    return out.reshape(1, D, H_BEV, W_BEV)


# revision 21
# speedup vs baseline: 1.1215x; 1.1215x over previous
"""Deformable cross-attention Trainium2 kernel (8-core SPMD, query-sharded).

Strategy (v2)
-------------
q_len = 64*64 = 4096 BEV queries split across 8 cores (512 each).  Per core:
  1. kv conv (PE, fp32) -> bf16 "kv2" scratch per camera in HBM:
     row r = y*88+x holds 1024 ch = [row r: k 256 | v 256][row r+88: k | v],
     i.e. the y+1 row is stacked channel-wise so ONE 4KB gather element
     (2 consecutive x positions) fetches the whole 2x2 bilinear footprint.
  2. All projections / offsets / coords / gather indices for all
     4 chunks x 6 cams computed upfront in a few batched DVE ops.
  3. Per (cam, chunk): dma_gather 1024 elements (128 q x 8 pts) of 4KB,
     then attention in bf16: q.k products (2x DVE mode), contiguous
     halving tree-adds for the dh-reduction (channels stored d-major so
     head lanes stay innermost/packed), score bilinear interp, softmax
     over points, a4 = att*wx*wy/6 folded weights, v weighted tree-sum.
  4. Output projection on PE per chunk.
Channels are permuted d-major (j = d*8+m <- m*32+d) host-side in
q_w/kv_w/proj_w so device reductions over d are contiguous halves.
No collectives; host concatenates the 8 query slices.

Free-dim biases q_b, kv_b, off_b2 are zeros per spec and not applied.
"""

import sys

for _p in ("/opt/trn_rl_repo", "/opt/trn_rl_repo/concourse"):
    if _p not in sys.path:
        sys.path.insert(0, _p)

from contextlib import ExitStack

import numpy as np

import concourse.bass as bass
import concourse.mybir as mybir
import concourse.tile as tile
from concourse import bacc, library_config
from concourse.bass_utils import run_bass_kernel_spmd

F32 = mybir.dt.float32
BF16 = mybir.dt.bfloat16
I16 = mybir.dt.int16
ALU = mybir.AluOpType
ACTF = mybir.ActivationFunctionType
AX = mybir.AxisListType

N_CORES = 8
D = 128
N_CAM = 6
H_BEV, W_BEV = 64, 64
Q_LEN = H_BEV * W_BEV            # 4096
QC = Q_LEN // N_CORES            # 512
N_CHUNK = QC // 128              # 4
HEADS, DH, NPTS = 8, 32, 8
INNER = HEADS * DH               # 256
HI, WI = 32, 88
POS = HI * WI                    # 2816
NPB = POS // 128                 # 22
CH2 = 1024                       # stacked kv2 channels per row
NIT = N_CHUNK * N_CAM            # 24 (cam, chunk) pairs

_PROGRAM = None


def _build_program():
    nc = bacc.Bacc("TRN2", target_bir_lowering=False, debug=False,
                   num_swdge_queues=2)

    # ---------------- I/O ----------------
    t_bev = nc.dram_tensor("bev_s", [D, QC], F32, kind="ExternalInput")
    t_world = nc.dram_tensor("world_s", [4, QC], F32, kind="ExternalInput")
    t_img = nc.dram_tensor("img", [N_CAM, D, POS], F32, kind="ExternalInput")
    t_mt = nc.dram_tensor("MT", [4, 3 * N_CAM], F32, kind="ExternalInput")
    t_w1T = nc.dram_tensor("w1T", [D, D], F32, kind="ExternalInput")
    t_w2T = nc.dram_tensor("w2T", [D, 2 * NPTS], F32, kind="ExternalInput")
    t_qwT = nc.dram_tensor("qwT", [D, INNER], F32, kind="ExternalInput")
    t_kvwT = nc.dram_tensor("kvwT", [D, 2 * INNER], F32, kind="ExternalInput")
    t_pwT = nc.dram_tensor("pwT", [128, 2, D], F32, kind="ExternalInput")
    t_b1 = nc.dram_tensor("b1", [D, 1], F32, kind="ExternalInput")
    t_pb = nc.dram_tensor("pb", [D, 1], F32, kind="ExternalInput")
    t_sel = nc.dram_tensor("selW", [128, 128], F32, kind="ExternalInput")
    t_mask = nc.dram_tensor("maskW", [128, 8], F32, kind="ExternalInput")
    t_idn = nc.dram_tensor("idn", [128, 128], F32, kind="ExternalInput")
    t_out = nc.dram_tensor("out", [D, QC], F32, kind="ExternalOutput")

    with tile.TileContext(nc) as tc, ExitStack() as ctx:
        nc.gpsimd.load_library(library_config.mlp)

        consts = ctx.enter_context(tc.tile_pool(name="consts", bufs=1))
        setupp = ctx.enter_context(tc.tile_pool(name="setup", bufs=1))
        drampool = ctx.enter_context(tc.tile_pool(name="dram", bufs=1, space="DRAM"))
        psA = ctx.enter_context(tc.tile_pool(name="psA", bufs=3, space="PSUM"))
        p2ps = psA
        p1ps = psA
        outps = psA

        def load_const(t, shape):
            s = consts.tile(shape, F32, tag=t.name)
            nc.sync.dma_start(s[:], t.ap())
            return s

        mt_all = load_const(t_mt, [4, 3 * N_CAM])
        xyz = load_const(t_world, [4, QC])
        c_bev = load_const(t_bev, [D, QC])
        c_w1T = load_const(t_w1T, [D, D])
        c_b1 = load_const(t_b1, [D, 1])
        c_w2T = load_const(t_w2T, [D, 2 * NPTS])
        c_kvwT = load_const(t_kvwT, [D, 2 * INNER])
        c_mask = load_const(t_mask, [128, 8])
        c_sel = load_const(t_sel, [128, 128])
        c_qwT = load_const(t_qwT, [D, INNER])
        c_pwT = load_const(t_pwT, [128, 2, D])
        c_pb = load_const(t_pb, [D, 1])
        c_idn = load_const(t_idn, [128, 128])

        kv2 = [drampool.tile([POS, CH2], BF16, tag=f"kv2_{n}", name=f"kv2_{n}")
               for n in range(N_CAM)]

        # ---------------- P2a: PE projections ----------------
        xh = setupp.tile([D, QC], F32, tag="xh_masked")
        pix_all = setupp.tile([128, N_CHUNK, 3 * N_CAM], F32)
        offT_all = setupp.tile([128, N_CHUNK, 2 * NPTS], F32)
        qT_all = setupp.tile([128, N_CHUNK, INNER], BF16)

        ps_xh = p2ps.tile([D, QC], F32, tag="big")
        nc.tensor.matmul(ps_xh[:], c_w1T[:], c_bev[:], start=True, stop=True)
        nc.scalar.activation(xh[:], ps_xh[:], ACTF.Relu, bias=c_b1[:])
        for c in range(N_CHUNK):
            cs = slice(c * 128, (c + 1) * 128)
            ps_pix = p2ps.tile([128, 3 * N_CAM], F32, tag="sm")
            nc.tensor.matmul(ps_pix[:], xyz[:, cs], mt_all[:], start=True, stop=True)
            nc.scalar.copy(pix_all[:, c, :], ps_pix[:])
        for c in range(N_CHUNK):
            cs = slice(c * 128, (c + 1) * 128)
            ps_o = p2ps.tile([128, 2 * NPTS], F32, tag="sm")
            nc.tensor.matmul(ps_o[:], xh[:, cs], c_w2T[:], start=True, stop=True)
            nc.scalar.copy(offT_all[:, c, :], ps_o[:])
        for c in range(N_CHUNK):
            cs = slice(c * 128, (c + 1) * 128)
            ps_q = p2ps.tile([128, INNER], F32, tag="sm")
            nc.tensor.matmul(ps_q[:], c_bev[:, cs], c_qwT[:], start=True, stop=True)
            nc.scalar.copy(qT_all[:, c, :], ps_q[:])

        # ---------------- P1: kv conv -> bf16 kv2 scratch ----------------
        p1pool = ctx.enter_context(tc.tile_pool(name="p1", bufs=1))
        CONV_PIECES = [range(0, 8), range(8, 16), range(16, NPB)]

        def conv_load(n, engine=None):
            img_t = p1pool.tile([D, POS], F32, tag="img", name="img_t", bufs=2)
            (engine or nc.sync).dma_start(img_t[:], t_img.ap()[n])
            return img_t

        def conv_piece(n, img_t, blocks):
            # compute blocks [b0, b1) and store them to kv2[n] immediately
            b0, b1 = blocks.start, blocks.stop
            nb = b1 - b0
            stg = p1pool.tile([128, 8, 2 * INNER], BF16, tag="stg", name="stg", bufs=2)
            for k, pb in enumerate(blocks):
                ps = p1ps.tile([128, 2 * INNER], F32, tag="big", name="ps")
                nc.tensor.matmul(
                    ps[:], img_t[:, pb * 128:(pb + 1) * 128], c_kvwT[:],
                    start=True, stop=True)
                nc.scalar.copy(stg[:, k, :], ps[:])
            # rows r=pb*128+p -> kv2[r, 0:512]
            dst = bass.AP(kv2[n][:].tensor, b0 * 128 * CH2,
                          [[CH2, 128], [128 * CH2, nb], [1, 512]])
            nc.sync.dma_start(dst, stg[:, 0:nb, :])
            # shifted copy: kv2[r-88, 512:1024] = row r  (r >= 88)
            if b0 == 0:
                dst_a = bass.AP(kv2[n][:].tensor, 512, [[CH2, 40], [1, 512]])
                nc.sync.dma_start(dst_a, stg[88:128, 0, :])
                dst_b = bass.AP(kv2[n][:].tensor, 40 * CH2 + 512,
                                [[CH2, 128], [128 * CH2, nb - 1], [1, 512]])
                nc.sync.dma_start(dst_b, stg[:, 1:nb, :])
            else:
                dst_b = bass.AP(kv2[n][:].tensor, (b0 * 128 - 88) * CH2 + 512,
                                [[CH2, 128], [128 * CH2, nb], [1, 512]])
                nc.sync.dma_start(dst_b, stg[:, 0:nb, :])

        img_m = conv_load(0)
        img1 = conv_load(1)
        for piece in CONV_PIECES:
            conv_piece(0, img_m, piece)

        # ---------------- P2b: coords / indices (batched DVE) ----------------
        NCN = N_CHUNK * N_CAM            # 24
        NQP = NCN * NPTS                 # 192
        sm24 = setupp.tile([128, 2 * NCN], F32)      # [zr rz | ux uy | gx gy]
        gx = setupp.tile([128, NCN], F32)
        gy = setupp.tile([128, NCN], F32)
        xw = setupp.tile([128, NQP], F32, tag="xw_shared")
        xs = setupp.tile([128, NQP], F32, tag="xs_shared")
        x0f = setupp.tile([128, NQP], F32)
        wxp = setupp.tile([128, NQP], F32)
        yw = setupp.tile([128, NQP], F32, tag="xw_shared")
        ys_ = setupp.tile([128, NQP], F32, tag="xs_shared")
        y0f = setupp.tile([128, NQP], F32)
        wyp = setupp.tile([128, NQP], F32)
        gtt = setupp.tile([128, NQP], F32)
        i16t = setupp.tile([128, NQP], I16)
        wx2 = setupp.tile([128, NQP, 2], F32)
        wy2 = setupp.tile([128, NQP, 2], F32)
        wxyk = setupp.tile([128, NQP, 2, 2], F32)
        wxy = setupp.tile([128, NQP, 2, 2], F32)
        i128 = setupp.tile([128, NQP], F32)
        masked = setupp.tile([128, 8, NPTS, 8], F32, tag="xh_masked")
        wrapped = setupp.tile([128, NCN, 64], I16)

        pixv = pix_all[:].rearrange("P c (n k) -> P c n k", n=N_CAM)
        zr = sm24[:, 0:NCN].rearrange("P (c n) -> P c n", c=N_CHUNK)
        rz = sm24[:, NCN:2 * NCN].rearrange("P (c n) -> P c n", c=N_CHUNK)
        nc.vector.tensor_scalar_max(zr, pixv[:, :, :, 2], 1e-6)
        nc.vector.reciprocal(rz, zr)
        gxv = gx[:].rearrange("P (c n) -> P c n", c=N_CHUNK)
        gyv = gy[:].rearrange("P (c n) -> P c n", c=N_CHUNK)
        nc.vector.tensor_mul(gxv, pixv[:, :, :, 0], rz)
        nc.vector.tensor_scalar(gxv, gxv, 2.0 / (WI - 1), -1.0, ALU.mult, ALU.add)
        nc.vector.tensor_mul(gyv, pixv[:, :, :, 1], rz)
        nc.vector.tensor_scalar(gyv, gyv, 2.0 / (HI - 1), -1.0, ALU.mult, ALU.add)

        offv = offT_all[:].rearrange("P c (p a) -> P c a p", a=2)

        def coord_chain(g_t, off_ax, w_t, s_t, f0_t, wfrac_t, hi_clip, scale):
            # w = clip(g + off, -1, 1) * scale + scale ; floor/clamp -> f0, frac
            wv = w_t[:].rearrange("P (c n p) -> P c n p", c=N_CHUNK, n=N_CAM)
            gb = g_t[:].rearrange("P (c n) -> P c n", c=N_CHUNK) \
                .unsqueeze(3).broadcast_to((128, N_CHUNK, N_CAM, NPTS))
            ob = offv[:, :, off_ax, :].unsqueeze(2) \
                .broadcast_to((128, N_CHUNK, N_CAM, NPTS))
            nc.vector.tensor_tensor(wv, gb, ob, ALU.add)
            nc.vector.tensor_scalar_min(w_t[:], w_t[:], 1.0)
            nc.vector.tensor_scalar_max(w_t[:], w_t[:], -1.0)
            nc.vector.tensor_scalar(w_t[:], w_t[:], scale, scale, ALU.mult, ALU.add)
            nc.vector.tensor_scalar_min(s_t[:], w_t[:], hi_clip)
            nc.vector.tensor_copy(i16t[:], s_t[:])
            nc.vector.tensor_copy(f0_t[:], i16t[:])
            nc.vector.tensor_tensor(gtt[:], f0_t[:], s_t[:], ALU.is_gt)
            nc.vector.tensor_sub(f0_t[:], f0_t[:], gtt[:])
            nc.vector.tensor_sub(wfrac_t[:], w_t[:], f0_t[:])

        coord_chain(gx, 0, xw, xs, x0f, wxp, float(WI - 2) + 0.5, (WI - 1) / 2.0)
        coord_chain(gy, 1, yw, ys_, y0f, wyp, float(HI - 2) + 0.5, (HI - 1) / 2.0)

        # gather row index = y0*88 + x0 (camera-local)
        nc.vector.tensor_scalar(i128[:], y0f[:], float(WI), 0.0, ALU.mult, ALU.add)
        nc.vector.tensor_add(i128[:], i128[:], x0f[:])

        # wrap indices for SWDGE: wrapped[r, it, pt*8+c8] = i128[c8*16+r, it, pt]
        i128v = i128[:].rearrange("P (i p) -> P i p", i=NCN)
        wflat = wrapped[:].rearrange("P i w -> P (i w)")
        for b in range(3):
            nc.vector.tensor_mul(
                masked[:],
                i128v[:, b * 8:(b + 1) * 8, :]
                .unsqueeze(3).broadcast_to((128, 8, NPTS, 8)),
                c_mask[:].unsqueeze(1).unsqueeze(2)
                .broadcast_to((128, 8, NPTS, 8)))
            ps_w = p2ps.tile([128, 512], F32, tag="big")
            nc.tensor.matmul(
                ps_w[:], c_sel[:],
                masked[:].rearrange("P i p e -> P (i p e)"),
                start=True, stop=True)
            nc.vector.tensor_copy(wflat[:, b * 512:(b + 1) * 512], ps_w[:])

        # corner weight products (1/N_CAM folded into wxy)
        nc.vector.tensor_scalar(wx2[:, :, 0], wxp[:], -1.0, 1.0, ALU.mult, ALU.add)
        nc.vector.tensor_copy(wx2[:, :, 1], wxp[:])
        nc.vector.tensor_scalar(wy2[:, :, 0], wyp[:], -1.0, 1.0, ALU.mult, ALU.add)
        nc.vector.tensor_copy(wy2[:, :, 1], wyp[:])
        nc.vector.tensor_mul(
            wxyk[:],
            wx2[:].unsqueeze(3).broadcast_to((128, NQP, 2, 2)),
            wy2[:].unsqueeze(2).broadcast_to((128, NQP, 2, 2)))
        nc.vector.tensor_scalar(wxy[:], wxyk[:], 1.0 / N_CAM, 0.0, ALU.mult, ALU.add)

        for piece in CONV_PIECES:
            conv_piece(1, img1, piece)

        # ---------------- P3: gather + attention ----------------
        gpool = ctx.enter_context(tc.tile_pool(name="G", bufs=3))
        kpool = ctx.enter_context(tc.tile_pool(name="kv", bufs=1))
        spool = ctx.enter_context(tc.tile_pool(name="small", bufs=2))
        accp = ctx.enter_context(tc.tile_pool(name="acc", bufs=1))

        accs = [accp.tile([128, INNER], F32, tag=f"acc{c}", name=f"acc{c}")
                for c in range(N_CHUNK)]
        wyv_all = wyp[:].rearrange("P (c n p) -> P c n p", c=N_CHUNK, n=N_CAM)
        wxv_all = wxp[:].rearrange("P (c n p) -> P c n p", c=N_CHUNK, n=N_CAM)
        wxyv_all = wxy[:].rearrange(
            "P (c n p) a b -> P c n (p a b)", c=N_CHUNK, n=N_CAM)
        wxyk_all = wxyk[:].rearrange(
            "P (c n p) a b -> P c n (p a b)", c=N_CHUNK, n=N_CAM)

        nxt = [None]

        for n in range(N_CAM):
            kv_src = bass.AP(kv2[n][:].tensor, 0, [[CH2, POS - 1], [1, 2048]])
            for c in range(N_CHUNK):
                it = c * N_CAM + n
                g = gpool.tile([128, NPTS, 2048], BF16, tag="G")
                nc.gpsimd.dma_gather(
                    g[:], kv_src,
                    wrapped[:, it, :], 1024, 1024,
                    elem_size=2048, elem_step=CH2, single_packet=True,
                    queue_num=it % 2)
                gkv = g[:].rearrange(
                    "P b (x y k i) -> P (b x y) k i", x=2, y=2, k=2)
                # ---- k side: prod = k * q, tree-reduce over d ----
                prod = kpool.tile([128, 32, INNER], BF16, tag="prod")
                nc.vector.tensor_mul(
                    prod[:], gkv[:, :, 0, :],
                    qT_all[:, c, :].unsqueeze(1).broadcast_to((128, 32, INNER)))
                t1 = kpool.tile([128, 32, 128], BF16, tag="t1")
                nc.vector.tensor_add(t1[:], prod[:, :, 0:128], prod[:, :, 128:256])
                t2 = kpool.tile([128, 32, 64], BF16, tag="t2")
                nc.vector.tensor_add(t2[:], t1[:, :, 0:64], t1[:, :, 64:128])
                t3 = kpool.tile([128, 32, 32], BF16, tag="t3")
                nc.vector.tensor_add(t3[:], t2[:, :, 0:32], t2[:, :, 32:64])
                t4 = kpool.tile([128, 32, 16], BF16, tag="t4")
                nc.vector.tensor_add(t4[:], t3[:, :, 0:16], t3[:, :, 16:32])

                # ---- bilinear interp of corner scores (weighted sum),
                #      fused with the last d-pair reduction level ----
                sims = kpool.tile([128, 32, 16], BF16, tag="sims")
                nc.vector.tensor_mul(
                    sims[:].rearrange("P s (d m) -> P s d m", m=HEADS),
                    t4[:].rearrange("P s (d m) -> P s d m", m=HEADS),
                    wxyk_all[:, c, n, :].unsqueeze(2).unsqueeze(3)
                    .broadcast_to((128, 32, 2, HEADS)))
                sim = spool.tile([128, NPTS, HEADS], F32, tag="sim")
                nc.vector.tensor_reduce(
                    sim[:],
                    sims[:].rearrange("P (p s) (d m) -> P p m (s d)",
                                      s=4, m=HEADS),
                    AX.X, ALU.add)

                # ---- softmax over points (scores bounded, skip max-sub) ----
                ev = spool.tile([128, NPTS, HEADS], F32, tag="ev")
                nc.scalar.activation(ev[:], sim[:], ACTF.Exp)
                ssum = spool.tile([128, HEADS], F32, tag="ssum")
                nc.vector.tensor_reduce(
                    ssum[:], ev[:].transpose([0, 2, 1]), AX.X, ALU.add)
                rr = spool.tile([128, HEADS], F32, tag="rr")
                nc.vector.reciprocal(rr[:], ssum[:])
                att = spool.tile([128, NPTS, HEADS], F32, tag="att")
                nc.vector.tensor_mul(
                    att[:], ev[:],
                    rr[:].unsqueeze(1).broadcast_to((128, NPTS, HEADS)))

                # ---- a4 = att * wx * wy / n  (bf16, [q, slot, m]) ----
                a4 = spool.tile([128, 32, HEADS], BF16, tag="a4")
                a4v = a4[:].rearrange("P (p s) m -> P p s m", s=4)
                nc.vector.tensor_mul(
                    a4v,
                    att[:].unsqueeze(2).broadcast_to((128, NPTS, 4, HEADS)),
                    wxyv_all[:, c, n, :].rearrange("P (p s) -> P p s", p=NPTS)
                    .unsqueeze(3).broadcast_to((128, NPTS, 4, HEADS)))

                # ---- v side: weighted tree-sum over 32 corner slots ----
                prodv = kpool.tile([128, 32, INNER], BF16, tag="prod")
                nc.vector.tensor_mul(
                    prodv[:].rearrange("P s (d m) -> P s d m", m=HEADS),
                    gkv[:, :, 1, :].rearrange("P s (d m) -> P s d m", m=HEADS),
                    a4[:].unsqueeze(2).broadcast_to((128, 32, DH, HEADS)))
                v1 = kpool.tile([128, 16, INNER], BF16, tag="t1")
                nc.vector.tensor_add(v1[:], prodv[:, 0:16, :], prodv[:, 16:32, :])
                v2 = kpool.tile([128, 8, INNER], BF16, tag="t2")
                nc.vector.tensor_add(v2[:], v1[:, 0:8, :], v1[:, 8:16, :])
                v3 = kpool.tile([128, 4, INNER], BF16, tag="t3")
                nc.vector.tensor_add(v3[:], v2[:, 0:4, :], v2[:, 4:8, :])
                v4 = kpool.tile([128, 2, INNER], BF16, tag="t4")
                nc.vector.tensor_add(v4[:], v3[:, 0:2, :], v3[:, 2:4, :])
                if n == 0:
                    nc.vector.tensor_add(accs[c][:], v4[:, 0, :], v4[:, 1, :])
                else:
                    v5 = kpool.tile([128, INNER], F32, tag="simc")
                    nc.vector.tensor_add(v5[:], v4[:, 0, :], v4[:, 1, :])
                    nc.vector.tensor_add(accs[c][:], accs[c][:], v5[:])

                # trickle cam n+2's conv with a full-camera lead so its
                # kv2 stores land well before that camera's gathers (emitted
                # after the attention body so exp precedes the copies in the
                # Scalar queue)
                if n + 2 < N_CAM and c < 3:
                    if c == 0:
                        nxt[0] = conv_load(n + 2, engine=nc.gpsimd)
                    conv_piece(n + 2, nxt[0], CONV_PIECES[c])

                # ---- P4: output projection, interleaved on the last cam ----
                if n == N_CAM - 1:
                    ps_out = outps.tile([128, 128], F32, tag="big")
                    for hh in range(2):
                        ps_tr = outps.tile([128, 128], F32, tag="sm")
                        nc.tensor.transpose(
                            ps_tr[:], accs[c][:, hh * 128:(hh + 1) * 128],
                            c_idn[:])
                        accT = kpool.tile([128, 128], F32, tag="t1")
                        nc.scalar.copy(accT[:], ps_tr[:])
                        nc.tensor.matmul(
                            ps_out[:], c_pwT[:, hh, :], accT[:],
                            start=(hh == 0), stop=(hh == 1))
                    out_sb = kpool.tile([128, 128], F32, tag="t2")
                    nc.vector.tensor_scalar_add(out_sb[:], ps_out[:], c_pb[:])
                    nc.sync.dma_start(
                        t_out.ap()[:, c * 128:(c + 1) * 128], out_sb[:])

    nc.compile()
    return nc


def _get_program():
    global _PROGRAM
    if _PROGRAM is None:
        _PROGRAM = _build_program()
    return _PROGRAM


def _host_inputs(inputs):
    bev = np.asarray(inputs["bev"], np.float32)
    img_feats = np.asarray(inputs["img_feats"], np.float32)
    K = np.asarray(inputs["K"], np.float32)
    E = np.asarray(inputs["E"], np.float32)
    world_xy = np.asarray(inputs["world_xy"], np.float32)

    bev2 = np.ascontiguousarray(bev.reshape(D, Q_LEN))
    world2 = np.ascontiguousarray(world_xy.reshape(2, Q_LEN))
    img = np.ascontiguousarray(img_feats.reshape(N_CAM, D, POS))
    # MT[n] = (K[n] @ E[n][:3, :]).T : [4, 3] per camera, concatenated
    mt = np.einsum('nij,njk->nik', K[0], E[0][:, :3, :])  # [n, 3, 4]
    mt = np.ascontiguousarray(mt.transpose(2, 0, 1).reshape(4, 3 * N_CAM))

    # d-major channel permutation: new j = d*8 + m  <-  old m*32 + d
    j = np.arange(INNER)
    pm = (j % HEADS) * DH + j // HEADS

    w1T = np.ascontiguousarray(np.asarray(inputs["off_w1"], np.float32).T)
    w2T = np.ascontiguousarray(np.asarray(inputs["off_w2"], np.float32).T)
    qwT = np.asarray(inputs["q_w"], np.float32).T
    qwT = np.ascontiguousarray(qwT[:, pm])
    kvwT = np.asarray(inputs["kv_w"], np.float32).T
    kvwT = np.ascontiguousarray(
        np.concatenate([kvwT[:, :INNER][:, pm], kvwT[:, INNER:][:, pm]], axis=1))
    pwTp = np.asarray(inputs["proj_w"], np.float32).T[pm]
    pwT = np.ascontiguousarray(pwTp.reshape(2, 128, 128).transpose(1, 0, 2))
    b1 = np.ascontiguousarray(np.asarray(inputs["off_b1"], np.float32).reshape(D, 1))
    pb = np.ascontiguousarray(np.asarray(inputs["proj_b"], np.float32).reshape(D, 1))

    kk = np.arange(128)
    sel = (kk[:, None] % 16 == kk[None, :] % 16).astype(np.float32)
    mask = (kk[:, None] // 16 == np.arange(8)[None, :]).astype(np.float32)
    idn = np.eye(128, dtype=np.float32)

    shared = dict(img=img, MT=mt, w1T=w1T, w2T=w2T, qwT=qwT, kvwT=kvwT,
                  pwT=pwT, b1=b1, pb=pb, selW=sel, maskW=mask, idn=idn)
    maps = []
    for r in range(N_CORES):
        s = slice(r * QC, (r + 1) * QC)
        m = dict(shared)
        m["bev_s"] = np.ascontiguousarray(bev2[:, s])
        ws = np.empty((4, QC), np.float32)
        ws[0:2] = world2[:, s]
        ws[2] = 0.0
        ws[3] = 1.0
        m["world_s"] = ws
        maps.append(m)
    return maps


def kernel(**inputs) -> np.ndarray:
    nc = _get_program()
    maps = _host_inputs(inputs)
    res = run_bass_kernel_spmd(nc, maps, list(range(N_CORES)))
    out = np.concatenate([res.results[r]["out"] for r in range(N_CORES)], axis=1)
    return out.reshape(1, D, H_BEV, W_BEV)


# revision 22
# speedup vs baseline: 1.1241x; 1.0023x over previous
"""Deformable cross-attention Trainium2 kernel (8-core SPMD, query-sharded).

Strategy (v2)
-------------
q_len = 64*64 = 4096 BEV queries split across 8 cores (512 each).  Per core:
  1. kv conv (PE, fp32) -> bf16 "kv2" scratch per camera in HBM:
     row r = y*88+x holds 1024 ch = [row r: k 256 | v 256][row r+88: k | v],
     i.e. the y+1 row is stacked channel-wise so ONE 4KB gather element
     (2 consecutive x positions) fetches the whole 2x2 bilinear footprint.
  2. All projections / offsets / coords / gather indices for all
     4 chunks x 6 cams computed upfront in a few batched DVE ops.
  3. Per (cam, chunk): dma_gather 1024 elements (128 q x 8 pts) of 4KB,
     then attention in bf16: q.k products (2x DVE mode), contiguous
     halving tree-adds for the dh-reduction (channels stored d-major so
     head lanes stay innermost/packed), score bilinear interp, softmax
     over points, a4 = att*wx*wy/6 folded weights, v weighted tree-sum.
  4. Output projection on PE per chunk.
Channels are permuted d-major (j = d*8+m <- m*32+d) host-side in
q_w/kv_w/proj_w so device reductions over d are contiguous halves.
No collectives; host concatenates the 8 query slices.

Free-dim biases q_b, kv_b, off_b2 are zeros per spec and not applied.
"""

import sys

for _p in ("/opt/trn_rl_repo", "/opt/trn_rl_repo/concourse"):
    if _p not in sys.path:
        sys.path.insert(0, _p)

from contextlib import ExitStack

import numpy as np

import concourse.bass as bass
import concourse.mybir as mybir
import concourse.tile as tile
from concourse import bacc, library_config
from concourse.bass_utils import run_bass_kernel_spmd

F32 = mybir.dt.float32
BF16 = mybir.dt.bfloat16
I16 = mybir.dt.int16
ALU = mybir.AluOpType
ACTF = mybir.ActivationFunctionType
AX = mybir.AxisListType

N_CORES = 8
D = 128
N_CAM = 6
H_BEV, W_BEV = 64, 64
Q_LEN = H_BEV * W_BEV            # 4096
QC = Q_LEN // N_CORES            # 512
N_CHUNK = QC // 128              # 4
HEADS, DH, NPTS = 8, 32, 8
INNER = HEADS * DH               # 256
HI, WI = 32, 88
POS = HI * WI                    # 2816
NPB = POS // 128                 # 22
CH2 = 1024                       # stacked kv2 channels per row
NIT = N_CHUNK * N_CAM            # 24 (cam, chunk) pairs

_PROGRAM = None


def _build_program():
    nc = bacc.Bacc("TRN2", target_bir_lowering=False, debug=False,
                   num_swdge_queues=2)

    # ---------------- I/O ----------------
    t_bev = nc.dram_tensor("bev_s", [D, QC], F32, kind="ExternalInput")
    t_world = nc.dram_tensor("world_s", [4, QC], F32, kind="ExternalInput")
    t_img = nc.dram_tensor("img", [N_CAM, D, POS], F32, kind="ExternalInput")
    t_mt = nc.dram_tensor("MT", [4, 3 * N_CAM], F32, kind="ExternalInput")
    t_w1T = nc.dram_tensor("w1T", [D, D], F32, kind="ExternalInput")
    t_w2T = nc.dram_tensor("w2T", [D, 2 * NPTS], F32, kind="ExternalInput")
    t_qwT = nc.dram_tensor("qwT", [D, INNER], F32, kind="ExternalInput")
    t_kvwT = nc.dram_tensor("kvwT", [D, 2 * INNER], F32, kind="ExternalInput")
    t_pwT = nc.dram_tensor("pwT", [128, 2, D], F32, kind="ExternalInput")
    t_b1 = nc.dram_tensor("b1", [D, 1], F32, kind="ExternalInput")
    t_pb = nc.dram_tensor("pb", [D, 1], F32, kind="ExternalInput")
    t_sel = nc.dram_tensor("selW", [128, 128], F32, kind="ExternalInput")
    t_mask = nc.dram_tensor("maskW", [128, 8], F32, kind="ExternalInput")
    t_idn = nc.dram_tensor("idn", [128, 128], F32, kind="ExternalInput")
    t_out = nc.dram_tensor("out", [D, QC], F32, kind="ExternalOutput")

    with tile.TileContext(nc) as tc, ExitStack() as ctx:
        nc.gpsimd.load_library(library_config.mlp)

        consts = ctx.enter_context(tc.tile_pool(name="consts", bufs=1))
        setupp = ctx.enter_context(tc.tile_pool(name="setup", bufs=1))
        drampool = ctx.enter_context(tc.tile_pool(name="dram", bufs=1, space="DRAM"))
        psA = ctx.enter_context(tc.tile_pool(name="psA", bufs=3, space="PSUM"))
        p2ps = psA
        p1ps = psA
        outps = psA

        def load_const(t, shape):
            s = consts.tile(shape, F32, tag=t.name)
            nc.sync.dma_start(s[:], t.ap())
            return s

        mt_all = load_const(t_mt, [4, 3 * N_CAM])
        xyz = load_const(t_world, [4, QC])
        c_bev = load_const(t_bev, [D, QC])
        c_w1T = load_const(t_w1T, [D, D])
        c_b1 = load_const(t_b1, [D, 1])
        c_w2T = load_const(t_w2T, [D, 2 * NPTS])
        c_kvwT = load_const(t_kvwT, [D, 2 * INNER])
        c_mask = load_const(t_mask, [128, 8])
        c_sel = load_const(t_sel, [128, 128])
        c_qwT = load_const(t_qwT, [D, INNER])
        c_pwT = load_const(t_pwT, [128, 2, D])
        c_pb = load_const(t_pb, [D, 1])
        c_idn = load_const(t_idn, [128, 128])

        kv2 = [drampool.tile([POS, CH2], BF16, tag=f"kv2_{n}", name=f"kv2_{n}")
               for n in range(N_CAM)]

        # ---------------- P2a: PE projections ----------------
        xh = setupp.tile([D, QC], F32, tag="xh_masked")
        pix_all = setupp.tile([128, N_CHUNK, 3 * N_CAM], F32)
        offT_all = setupp.tile([128, N_CHUNK, 2 * NPTS], F32)
        qT_all = setupp.tile([128, N_CHUNK, INNER], BF16)

        ps_xh = p2ps.tile([D, QC], F32, tag="big")
        nc.tensor.matmul(ps_xh[:], c_w1T[:], c_bev[:], start=True, stop=True)
        nc.scalar.activation(xh[:], ps_xh[:], ACTF.Relu, bias=c_b1[:])
        for c in range(N_CHUNK):
            cs = slice(c * 128, (c + 1) * 128)
            ps_pix = p2ps.tile([128, 3 * N_CAM], F32, tag="sm")
            nc.tensor.matmul(ps_pix[:], xyz[:, cs], mt_all[:], start=True, stop=True)
            nc.vector.tensor_copy(pix_all[:, c, :], ps_pix[:])
        for c in range(N_CHUNK):
            cs = slice(c * 128, (c + 1) * 128)
            ps_o = p2ps.tile([128, 2 * NPTS], F32, tag="sm")
            nc.tensor.matmul(ps_o[:], xh[:, cs], c_w2T[:], start=True, stop=True)
            nc.vector.tensor_copy(offT_all[:, c, :], ps_o[:])
        for c in range(N_CHUNK):
            cs = slice(c * 128, (c + 1) * 128)
            ps_q = p2ps.tile([128, INNER], F32, tag="sm")
            nc.tensor.matmul(ps_q[:], c_bev[:, cs], c_qwT[:], start=True, stop=True)
            nc.vector.tensor_copy(qT_all[:, c, :], ps_q[:])

        # ---------------- P1: kv conv -> bf16 kv2 scratch ----------------
        p1pool = ctx.enter_context(tc.tile_pool(name="p1", bufs=1))
        CONV_PIECES = [range(0, 8), range(8, 16), range(16, NPB)]

        def conv_load(n, engine=None):
            img_t = p1pool.tile([D, POS], F32, tag="img", name="img_t", bufs=2)
            (engine or nc.sync).dma_start(img_t[:], t_img.ap()[n])
            return img_t

        def conv_piece(n, img_t, blocks):
            # compute blocks [b0, b1) and store them to kv2[n] immediately
            b0, b1 = blocks.start, blocks.stop
            nb = b1 - b0
            stg = p1pool.tile([128, 8, 2 * INNER], BF16, tag="stg", name="stg", bufs=2)
            for k, pb in enumerate(blocks):
                ps = p1ps.tile([128, 2 * INNER], F32, tag="big", name="ps")
                nc.tensor.matmul(
                    ps[:], img_t[:, pb * 128:(pb + 1) * 128], c_kvwT[:],
                    start=True, stop=True)
                nc.scalar.copy(stg[:, k, :], ps[:])
            # rows r=pb*128+p -> kv2[r, 0:512]
            dst = bass.AP(kv2[n][:].tensor, b0 * 128 * CH2,
                          [[CH2, 128], [128 * CH2, nb], [1, 512]])
            nc.sync.dma_start(dst, stg[:, 0:nb, :])
            # shifted copy: kv2[r-88, 512:1024] = row r  (r >= 88)
            if b0 == 0:
                dst_a = bass.AP(kv2[n][:].tensor, 512, [[CH2, 40], [1, 512]])
                nc.sync.dma_start(dst_a, stg[88:128, 0, :])
                dst_b = bass.AP(kv2[n][:].tensor, 40 * CH2 + 512,
                                [[CH2, 128], [128 * CH2, nb - 1], [1, 512]])
                nc.sync.dma_start(dst_b, stg[:, 1:nb, :])
            else:
                dst_b = bass.AP(kv2[n][:].tensor, (b0 * 128 - 88) * CH2 + 512,
                                [[CH2, 128], [128 * CH2, nb], [1, 512]])
                nc.sync.dma_start(dst_b, stg[:, 0:nb, :])

        img_m = conv_load(0)
        img1 = conv_load(1)
        for piece in CONV_PIECES:
            conv_piece(0, img_m, piece)

        # ---------------- P2b: coords / indices (batched DVE) ----------------
        NCN = N_CHUNK * N_CAM            # 24
        NQP = NCN * NPTS                 # 192
        sm24 = setupp.tile([128, 2 * NCN], F32)      # [zr rz | ux uy | gx gy]
        gx = setupp.tile([128, NCN], F32)
        gy = setupp.tile([128, NCN], F32)
        xw = setupp.tile([128, NQP], F32, tag="xw_shared")
        xs = setupp.tile([128, NQP], F32, tag="xs_shared")
        x0f = setupp.tile([128, NQP], F32)
        wxp = setupp.tile([128, NQP], F32)
        yw = setupp.tile([128, NQP], F32, tag="xw_shared")
        ys_ = setupp.tile([128, NQP], F32, tag="xs_shared")
        y0f = setupp.tile([128, NQP], F32)
        wyp = setupp.tile([128, NQP], F32)
        gtt = setupp.tile([128, NQP], F32)
        i16t = setupp.tile([128, NQP], I16)
        wx2 = setupp.tile([128, NQP, 2], F32)
        wy2 = setupp.tile([128, NQP, 2], F32)
        wxyk = setupp.tile([128, NQP, 2, 2], F32)
        wxy = setupp.tile([128, NQP, 2, 2], F32)
        i128 = setupp.tile([128, NQP], F32)
        masked = setupp.tile([128, 8, NPTS, 8], F32, tag="xh_masked")
        wrapped = setupp.tile([128, NCN, 64], I16)

        pixv = pix_all[:].rearrange("P c (n k) -> P c n k", n=N_CAM)
        zr = sm24[:, 0:NCN].rearrange("P (c n) -> P c n", c=N_CHUNK)
        rz = sm24[:, NCN:2 * NCN].rearrange("P (c n) -> P c n", c=N_CHUNK)
        nc.vector.tensor_scalar_max(zr, pixv[:, :, :, 2], 1e-6)
        nc.vector.reciprocal(rz, zr)
        gxv = gx[:].rearrange("P (c n) -> P c n", c=N_CHUNK)
        gyv = gy[:].rearrange("P (c n) -> P c n", c=N_CHUNK)
        nc.vector.tensor_mul(gxv, pixv[:, :, :, 0], rz)
        nc.vector.tensor_scalar(gxv, gxv, 2.0 / (WI - 1), -1.0, ALU.mult, ALU.add)
        nc.vector.tensor_mul(gyv, pixv[:, :, :, 1], rz)
        nc.vector.tensor_scalar(gyv, gyv, 2.0 / (HI - 1), -1.0, ALU.mult, ALU.add)

        offv = offT_all[:].rearrange("P c (p a) -> P c a p", a=2)

        def coord_chain(g_t, off_ax, w_t, s_t, f0_t, wfrac_t, hi_clip, scale):
            # w = clip(g + off, -1, 1) * scale + scale ; floor/clamp -> f0, frac
            wv = w_t[:].rearrange("P (c n p) -> P c n p", c=N_CHUNK, n=N_CAM)
            gb = g_t[:].rearrange("P (c n) -> P c n", c=N_CHUNK) \
                .unsqueeze(3).broadcast_to((128, N_CHUNK, N_CAM, NPTS))
            ob = offv[:, :, off_ax, :].unsqueeze(2) \
                .broadcast_to((128, N_CHUNK, N_CAM, NPTS))
            nc.vector.tensor_tensor(wv, gb, ob, ALU.add)
            nc.vector.tensor_scalar_min(w_t[:], w_t[:], 1.0)
            nc.vector.tensor_scalar_max(w_t[:], w_t[:], -1.0)
            nc.vector.tensor_scalar(w_t[:], w_t[:], scale, scale, ALU.mult, ALU.add)
            nc.vector.tensor_scalar_min(s_t[:], w_t[:], hi_clip)
            nc.vector.tensor_copy(i16t[:], s_t[:])
            nc.vector.tensor_copy(f0_t[:], i16t[:])
            nc.vector.tensor_tensor(gtt[:], f0_t[:], s_t[:], ALU.is_gt)
            nc.vector.tensor_sub(f0_t[:], f0_t[:], gtt[:])
            nc.vector.tensor_sub(wfrac_t[:], w_t[:], f0_t[:])

        coord_chain(gx, 0, xw, xs, x0f, wxp, float(WI - 2) + 0.5, (WI - 1) / 2.0)
        coord_chain(gy, 1, yw, ys_, y0f, wyp, float(HI - 2) + 0.5, (HI - 1) / 2.0)

        # gather row index = y0*88 + x0 (camera-local)
        nc.vector.tensor_scalar(i128[:], y0f[:], float(WI), 0.0, ALU.mult, ALU.add)
        nc.vector.tensor_add(i128[:], i128[:], x0f[:])

        # wrap indices for SWDGE: wrapped[r, it, pt*8+c8] = i128[c8*16+r, it, pt]
        i128v = i128[:].rearrange("P (i p) -> P i p", i=NCN)
        wflat = wrapped[:].rearrange("P i w -> P (i w)")
        for b in range(3):
            nc.vector.tensor_mul(
                masked[:],
                i128v[:, b * 8:(b + 1) * 8, :]
                .unsqueeze(3).broadcast_to((128, 8, NPTS, 8)),
                c_mask[:].unsqueeze(1).unsqueeze(2)
                .broadcast_to((128, 8, NPTS, 8)))
            ps_w = p2ps.tile([128, 512], F32, tag="big")
            nc.tensor.matmul(
                ps_w[:], c_sel[:],
                masked[:].rearrange("P i p e -> P (i p e)"),
                start=True, stop=True)
            nc.vector.tensor_copy(wflat[:, b * 512:(b + 1) * 512], ps_w[:])

        # corner weight products (1/N_CAM folded into wxy)
        nc.vector.tensor_scalar(wx2[:, :, 0], wxp[:], -1.0, 1.0, ALU.mult, ALU.add)
        nc.vector.tensor_copy(wx2[:, :, 1], wxp[:])
        nc.vector.tensor_scalar(wy2[:, :, 0], wyp[:], -1.0, 1.0, ALU.mult, ALU.add)
        nc.vector.tensor_copy(wy2[:, :, 1], wyp[:])
        nc.vector.tensor_mul(
            wxyk[:],
            wx2[:].unsqueeze(3).broadcast_to((128, NQP, 2, 2)),
            wy2[:].unsqueeze(2).broadcast_to((128, NQP, 2, 2)))
        nc.vector.tensor_scalar(wxy[:], wxyk[:], 1.0 / N_CAM, 0.0, ALU.mult, ALU.add)

        for piece in CONV_PIECES:
            conv_piece(1, img1, piece)

        # ---------------- P3: gather + attention ----------------
        gpool = ctx.enter_context(tc.tile_pool(name="G", bufs=3))
        kpool = ctx.enter_context(tc.tile_pool(name="kv", bufs=1))
        spool = ctx.enter_context(tc.tile_pool(name="small", bufs=2))
        accp = ctx.enter_context(tc.tile_pool(name="acc", bufs=1))

        accs = [accp.tile([128, INNER], F32, tag=f"acc{c}", name=f"acc{c}")
                for c in range(N_CHUNK)]
        wyv_all = wyp[:].rearrange("P (c n p) -> P c n p", c=N_CHUNK, n=N_CAM)
        wxv_all = wxp[:].rearrange("P (c n p) -> P c n p", c=N_CHUNK, n=N_CAM)
        wxyv_all = wxy[:].rearrange(
            "P (c n p) a b -> P c n (p a b)", c=N_CHUNK, n=N_CAM)
        wxyk_all = wxyk[:].rearrange(
            "P (c n p) a b -> P c n (p a b)", c=N_CHUNK, n=N_CAM)

        nxt = [None]

        for n in range(N_CAM):
            kv_src = bass.AP(kv2[n][:].tensor, 0, [[CH2, POS - 1], [1, 2048]])
            for c in range(N_CHUNK):
                it = c * N_CAM + n
                g = gpool.tile([128, NPTS, 2048], BF16, tag="G")
                nc.gpsimd.dma_gather(
                    g[:], kv_src,
                    wrapped[:, it, :], 1024, 1024,
                    elem_size=2048, elem_step=CH2, single_packet=True,
                    queue_num=it % 2)
                # trickle cam n+2's conv with a full-camera lead; emitted
                # early in the body so its copies drain during the k-mul phase
                if n + 2 < N_CAM and c < 3:
                    if c == 0:
                        nxt[0] = conv_load(n + 2, engine=nc.gpsimd)
                    conv_piece(n + 2, nxt[0], CONV_PIECES[c])

                gkv = g[:].rearrange(
                    "P b (x y k i) -> P (b x y) k i", x=2, y=2, k=2)
                # ---- k side: prod = k * q, tree-reduce over d ----
                prod = kpool.tile([128, 32, INNER], BF16, tag="prod")
                nc.vector.tensor_mul(
                    prod[:], gkv[:, :, 0, :],
                    qT_all[:, c, :].unsqueeze(1).broadcast_to((128, 32, INNER)))
                t1 = kpool.tile([128, 32, 128], BF16, tag="t1")
                nc.vector.tensor_add(t1[:], prod[:, :, 0:128], prod[:, :, 128:256])
                t2 = kpool.tile([128, 32, 64], BF16, tag="t2")
                nc.vector.tensor_add(t2[:], t1[:, :, 0:64], t1[:, :, 64:128])
                t3 = kpool.tile([128, 32, 32], BF16, tag="t3")
                nc.vector.tensor_add(t3[:], t2[:, :, 0:32], t2[:, :, 32:64])
                t4 = kpool.tile([128, 32, 16], BF16, tag="t4")
                nc.vector.tensor_add(t4[:], t3[:, :, 0:16], t3[:, :, 16:32])

                # ---- bilinear interp of corner scores (weighted sum),
                #      fused with the last d-pair reduction level ----
                sims = kpool.tile([128, 32, 16], BF16, tag="sims")
                nc.vector.tensor_mul(
                    sims[:].rearrange("P s (d m) -> P s d m", m=HEADS),
                    t4[:].rearrange("P s (d m) -> P s d m", m=HEADS),
                    wxyk_all[:, c, n, :].unsqueeze(2).unsqueeze(3)
                    .broadcast_to((128, 32, 2, HEADS)))
                sim = spool.tile([128, NPTS, HEADS], F32, tag="sim")
                nc.vector.tensor_reduce(
                    sim[:],
                    sims[:].rearrange("P (p s) (d m) -> P p m (s d)",
                                      s=4, m=HEADS),
                    AX.X, ALU.add)

                # ---- softmax over points (scores bounded, skip max-sub) ----
                ev = spool.tile([128, NPTS, HEADS], F32, tag="ev")
                nc.scalar.activation(ev[:], sim[:], ACTF.Exp)
                ssum = spool.tile([128, HEADS], F32, tag="ssum")
                nc.vector.tensor_reduce(
                    ssum[:], ev[:].transpose([0, 2, 1]), AX.X, ALU.add)
                rr = spool.tile([128, HEADS], F32, tag="rr")
                nc.vector.reciprocal(rr[:], ssum[:])
                att = spool.tile([128, NPTS, HEADS], F32, tag="att")
                nc.vector.tensor_mul(
                    att[:], ev[:],
                    rr[:].unsqueeze(1).broadcast_to((128, NPTS, HEADS)))

                # ---- a4 = att * wx * wy / n  (bf16, [q, slot, m]) ----
                a4 = spool.tile([128, 32, HEADS], BF16, tag="a4")
                a4v = a4[:].rearrange("P (p s) m -> P p s m", s=4)
                nc.vector.tensor_mul(
                    a4v,
                    att[:].unsqueeze(2).broadcast_to((128, NPTS, 4, HEADS)),
                    wxyv_all[:, c, n, :].rearrange("P (p s) -> P p s", p=NPTS)
                    .unsqueeze(3).broadcast_to((128, NPTS, 4, HEADS)))

                # ---- v side: weighted tree-sum over 32 corner slots ----
                prodv = kpool.tile([128, 32, INNER], BF16, tag="prod")
                nc.vector.tensor_mul(
                    prodv[:].rearrange("P s (d m) -> P s d m", m=HEADS),
                    gkv[:, :, 1, :].rearrange("P s (d m) -> P s d m", m=HEADS),
                    a4[:].unsqueeze(2).broadcast_to((128, 32, DH, HEADS)))
                v1 = kpool.tile([128, 16, INNER], BF16, tag="t1")
                nc.vector.tensor_add(v1[:], prodv[:, 0:16, :], prodv[:, 16:32, :])
                v2 = kpool.tile([128, 8, INNER], BF16, tag="t2")
                nc.vector.tensor_add(v2[:], v1[:, 0:8, :], v1[:, 8:16, :])
                v3 = kpool.tile([128, 4, INNER], BF16, tag="t3")
                nc.vector.tensor_add(v3[:], v2[:, 0:4, :], v2[:, 4:8, :])
                v4 = kpool.tile([128, 2, INNER], BF16, tag="t4")
                nc.vector.tensor_add(v4[:], v3[:, 0:2, :], v3[:, 2:4, :])
                if n == 0:
                    nc.vector.tensor_add(accs[c][:], v4[:, 0, :], v4[:, 1, :])
                else:
                    v5 = kpool.tile([128, INNER], F32, tag="simc")
                    nc.vector.tensor_add(v5[:], v4[:, 0, :], v4[:, 1, :])
                    nc.vector.tensor_add(accs[c][:], accs[c][:], v5[:])

                # ---- P4: output projection, interleaved on the last cam ----
                if n == N_CAM - 1:
                    ps_out = outps.tile([128, 128], F32, tag="big")
                    for hh in range(2):
                        ps_tr = outps.tile([128, 128], F32, tag="sm")
                        nc.tensor.transpose(
                            ps_tr[:], accs[c][:, hh * 128:(hh + 1) * 128],
                            c_idn[:])
                        accT = kpool.tile([128, 128], F32, tag="t1")
                        nc.scalar.copy(accT[:], ps_tr[:])
                        nc.tensor.matmul(
                            ps_out[:], c_pwT[:, hh, :], accT[:],
                            start=(hh == 0), stop=(hh == 1))
                    out_sb = kpool.tile([128, 128], F32, tag="t2")
                    nc.vector.tensor_scalar_add(out_sb[:], ps_out[:], c_pb[:])
                    nc.sync.dma_start(
                        t_out.ap()[:, c * 128:(c + 1) * 128], out_sb[:])

    nc.compile()
    return nc


def _get_program():
    global _PROGRAM
    if _PROGRAM is None:
        _PROGRAM = _build_program()
    return _PROGRAM


def _host_inputs(inputs):
    bev = np.asarray(inputs["bev"], np.float32)
    img_feats = np.asarray(inputs["img_feats"], np.float32)
    K = np.asarray(inputs["K"], np.float32)
    E = np.asarray(inputs["E"], np.float32)
    world_xy = np.asarray(inputs["world_xy"], np.float32)

    bev2 = np.ascontiguousarray(bev.reshape(D, Q_LEN))
    world2 = np.ascontiguousarray(world_xy.reshape(2, Q_LEN))
    img = np.ascontiguousarray(img_feats.reshape(N_CAM, D, POS))
    # MT[n] = (K[n] @ E[n][:3, :]).T : [4, 3] per camera, concatenated
    mt = np.einsum('nij,njk->nik', K[0], E[0][:, :3, :])  # [n, 3, 4]
    mt = np.ascontiguousarray(mt.transpose(2, 0, 1).reshape(4, 3 * N_CAM))

    # d-major channel permutation: new j = d*8 + m  <-  old m*32 + d
    j = np.arange(INNER)
    pm = (j % HEADS) * DH + j // HEADS

    w1T = np.ascontiguousarray(np.asarray(inputs["off_w1"], np.float32).T)
    w2T = np.ascontiguousarray(np.asarray(inputs["off_w2"], np.float32).T)
    qwT = np.asarray(inputs["q_w"], np.float32).T
    qwT = np.ascontiguousarray(qwT[:, pm])
    kvwT = np.asarray(inputs["kv_w"], np.float32).T
    kvwT = np.ascontiguousarray(
        np.concatenate([kvwT[:, :INNER][:, pm], kvwT[:, INNER:][:, pm]], axis=1))
    pwTp = np.asarray(inputs["proj_w"], np.float32).T[pm]
    pwT = np.ascontiguousarray(pwTp.reshape(2, 128, 128).transpose(1, 0, 2))
    b1 = np.ascontiguousarray(np.asarray(inputs["off_b1"], np.float32).reshape(D, 1))
    pb = np.ascontiguousarray(np.asarray(inputs["proj_b"], np.float32).reshape(D, 1))

    kk = np.arange(128)
    sel = (kk[:, None] % 16 == kk[None, :] % 16).astype(np.float32)
    mask = (kk[:, None] // 16 == np.arange(8)[None, :]).astype(np.float32)
    idn = np.eye(128, dtype=np.float32)

    shared = dict(img=img, MT=mt, w1T=w1T, w2T=w2T, qwT=qwT, kvwT=kvwT,
                  pwT=pwT, b1=b1, pb=pb, selW=sel, maskW=mask, idn=idn)
    maps = []
    for r in range(N_CORES):
        s = slice(r * QC, (r + 1) * QC)
        m = dict(shared)
        m["bev_s"] = np.ascontiguousarray(bev2[:, s])
        ws = np.empty((4, QC), np.float32)
        ws[0:2] = world2[:, s]
        ws[2] = 0.0
        ws[3] = 1.0
        m["world_s"] = ws
        maps.append(m)
    return maps


def kernel(**inputs) -> np.ndarray:
    nc = _get_program()
    maps = _host_inputs(inputs)
    res = run_bass_kernel_spmd(nc, maps, list(range(N_CORES)))
    out = np.concatenate([res.results[r]["out"] for r in range(N_CORES)], axis=1)
    return out.reshape(1, D, H_BEV, W_BEV)
